# revision 37
# baseline (speedup 1.0000x reference)
"""Fused attention block (RMSNorm -> QKV -> 2D RoPE -> SDPA -> proj) on 8
Trainium2 NeuronCores, data-parallel over the batch dimension (B=8, one batch
element per core; no collectives).

Self-contained: hardcodes shapes B=8, N=1024, C=1024, H=16, D=64.
"""

import numpy as np
import ml_dtypes

B = 8
N = 1024
C = 1024
H = 16
D = 64
GRID = 32
EPS = 1e-6
P = 128
NT = N // P      # 8 token tiles
CT = C // P      # 8 channel tiles

BF16 = ml_dtypes.bfloat16


def _rope_tables():
    """Feature-major RoPE tables [64, N] duplicated to [128, N].

    cosp[p, n] = cos(angle[n, (p % 64) // 2])
    sinp[p, n] = -sin(...) for even (p%64), +sin(...) for odd.
    """
    quarter = D // 4
    freqs = 1.0 / (10000.0 ** (np.arange(quarter, dtype=np.float64) / quarter))
    ys, xs = np.meshgrid(np.arange(GRID, dtype=np.float64),
                         np.arange(GRID, dtype=np.float64), indexing='ij')
    ang_x = xs.reshape(-1)[:, None] * freqs[None, :]      # [N, 16]
    ang_y = ys.reshape(-1)[:, None] * freqs[None, :]      # [N, 16]
    angles = np.concatenate([ang_x, ang_y], axis=-1)      # [N, 32]
    cos = np.cos(angles)
    sin = np.sin(angles)
    cos_d = np.repeat(cos, 2, axis=1).T                   # [64, N]
    sin_d = np.repeat(sin, 2, axis=1).T                   # [64, N]
    sign = np.where(np.arange(D) % 2 == 0, -1.0, 1.0)[:, None]
    sin_d = sin_d * sign
    cosp = np.concatenate([cos_d, cos_d], axis=0)         # [128, N]
    sinp = np.concatenate([sin_d, sin_d], axis=0)
    return cosp.astype(np.float32), sinp.astype(np.float32)


def build(nc):
    import concourse.mybir as mybir
    import concourse.tile as tile
    from concourse.masks import make_identity

    f32 = mybir.dt.float32
    bf16 = mybir.dt.bfloat16
    AF = mybir.ActivationFunctionType
    ALU = mybir.AluOpType

    x_ext = nc.declare_dram_parameter("x", [N, C], bf16, isOutput=False)
    # q|k weight columns, pre-tiled [f, j, 128c, 128col]
    wqk_ext = nc.declare_dram_parameter("wqk", [16, CT, P, P], bf16,
                                        isOutput=False)
    wv_ext = nc.declare_dram_parameter("wv", [C, C], bf16, isOutput=False)
    wproj_ext = nc.declare_dram_parameter("wproj", [C, C], bf16,
                                          isOutput=False)
    bproj_ext = nc.declare_dram_parameter("bproj", [1, C], bf16,
                                          isOutput=False)
    cosp_ext = nc.declare_dram_parameter("cosp", [P, N], bf16, isOutput=False)
    sinp_ext = nc.declare_dram_parameter("sinp", [P, N], bf16, isOutput=False)
    out_ext = nc.declare_dram_parameter("out", [N, C], f32, isOutput=True)

    from contextlib import ExitStack
    with tile.TileContext(nc) as tc:
        with ExitStack() as ctx:
            pool = lambda name, bufs, **kw: ctx.enter_context(
                tc.tile_pool(name=name, bufs=bufs, **kw))
            consts = pool("consts", 1)
            tabs = pool("tabs", 1)
            wv_pool = pool("wv", 1)
            wproj_pool = pool("wproj", 1)
            wqk_pool = pool("wqks", 2)
            xnt_pool = pool("xnt", 1)
            qk_pool = pool("qk", 3)
            v_pool = pool("vaug", 1)
            o_pool = pool("ofm", 1)
            x_pool = pool("xin", 2)
            st_pool = pool("stats", 4)
            sq_pool = pool("sq", 1)
            xn_pool = pool("xn", 2)
            qs_pool = pool("qsev", 2)
            tmp_pool = pool("tmp", 2)
            rt_pool = pool("ropet", 2)
            et_pool = pool("et", 2)
            r_pool = pool("rstat", 2)
            rb_pool = pool("rbc", 2)
            out_pool = pool("outst", 1)
            pm = pool("pm", 3, space="PSUM")
            ident = consts.tile([P, P], bf16, name="ident")
            make_identity(nc, ident[:])
            ones_col = consts.tile([1, P], bf16, name="ones_col")
            nc.vector.memset(ones_col[:], 1.0)
            eps_t = consts.tile([P, 1], f32, name="eps_t")
            nc.vector.memset(eps_t[:], EPS)
            bproj_sb = consts.tile([1, C], bf16, name="bproj_sb")
            nc.scalar.dma_start(bproj_sb[:], bproj_ext[:])

            cosp = tabs.tile([P, N], bf16, name="cosp")
            sinp = tabs.tile([P, N], bf16, name="sinp")
            nc.scalar.dma_start(cosp[:], cosp_ext[:])
            nc.scalar.dma_start(sinp[:], sinp_ext[:])

            xnt = [xnt_pool.tile([P, N], bf16, name=f"xnt{j}", tag=f"xnt{j}")
                   for j in range(CT)]
            v_aug = [v_pool.tile([P, H * (D + 1)], bf16, name=f"vaug{i}",
                                 tag=f"vaug{i}") for i in range(NT)]
            o_fm = [o_pool.tile([P, N], bf16, name=f"ofm{f}", tag=f"ofm{f}")
                    for f in range(CT)]

            # weights on the scalar queue: issue immediately, parallel to x
            wv_t = [wv_pool.tile([P, C], bf16, name=f"wv{j}", tag=f"wv{j}")
                    for j in range(CT)]
            for j in range(CT):
                nc.scalar.dma_start(wv_t[j][:], wv_ext[j * P:(j + 1) * P, :])
            wproj_t = [wproj_pool.tile([P, C], bf16, name=f"wp{j}",
                                       tag=f"wp{j}") for j in range(CT)]
            for j in range(CT):
                nc.scalar.dma_start(wproj_t[j][:],
                                    wproj_ext[j * P:(j + 1) * P, :])

            # ---- x load + RMSNorm + transpose ----------------------------
            for i in range(NT):
                x_i = x_pool.tile([P, C], bf16, name="x_i")
                nc.sync.dma_start(x_i[:], x_ext[i * P:(i + 1) * P, :])
                sq = sq_pool.tile([P, C], bf16, name="sq")
                ssq = st_pool.tile([P, 1], f32, name="ssq")
                nc.scalar.activation(sq[:], x_i[:], AF.Square,
                                     accum_out=ssq[:])
                rms = st_pool.tile([P, 1], f32, name="rms")
                nc.scalar.activation(rms[:], ssq[:], AF.Sqrt,
                                     scale=1.0 / C, bias=eps_t[:])
                rs = st_pool.tile([P, 1], f32, name="rs")
                nc.vector.reciprocal(rs[:], rms[:])
                xn = xn_pool.tile([P, C], bf16, name="xn")
                nc.vector.tensor_scalar(xn[:], x_i[:], rs[:], None, ALU.mult)
                for j in range(CT):
                    pt = pm.tile([P, P], bf16, name="pt", tag="mm", bufs=2)
                    nc.tensor.transpose(pt[:], xn[:, j * P:(j + 1) * P],
                                        ident[:])
                    nc.vector.tensor_copy(xnt[j][:, i * P:(i + 1) * P], pt[:])

            # ---- v token-major --------------------------------------------
            for i in range(NT):
                ps = pm.tile([P, N], f32, name="psv", tag="mm", bufs=2)
                for ch in range(2):
                    for j in range(CT):
                        nc.tensor.matmul(
                            ps[:, ch * 512:(ch + 1) * 512],
                            lhsT=xnt[j][:, i * P:(i + 1) * P],
                            rhs=wv_t[j][:, ch * 512:(ch + 1) * 512],
                            start=(j == 0), stop=(j == CT - 1),
                        )
                dst = v_aug[i].rearrange("p (h e) -> p h e", e=D + 1)
                nc.scalar.activation(
                    dst[:, :, 0:D], ps.rearrange("p (h d) -> p h d", d=D),
                    AF.Copy)
                nc.vector.memset(dst[:, :, D:D + 1], 1.0)

            # ---- interleaved q/k + attention per head-pair ----------------
            for p in range(CT):
                qk = {}
                for f in (p, 8 + p):
                    qk[f] = qk_pool.tile([P, N], bf16, name=f"qk{f}",
                                         tag="qkq" if f < 8 else "qkk")
                    wqk_s = [wqk_pool.tile([P, P], bf16, name=f"wqk{j}",
                                           tag=f"wqk{j}") for j in range(CT)]
                    for j in range(CT):
                        nc.sync.dma_start(wqk_s[j][:], wqk_ext[f, j])
                    ps = pm.tile([P, N], f32, name="psqk", tag="qkps", bufs=1)
                    for ch in range(2):
                        for j in range(CT):
                            nc.tensor.matmul(
                                ps[:, ch * 512:(ch + 1) * 512],
                                lhsT=wqk_s[j][:],
                                rhs=xnt[j][:, ch * 512:(ch + 1) * 512],
                                start=(j == 0), stop=(j == CT - 1),
                            )
                    qs = qs_pool.tile([P, N], bf16, name="qs")
                    nc.scalar.activation(qs[:], ps[:], AF.Copy)
                    tmp = tmp_pool.tile([P, N], bf16, name="tmp")
                    nc.gpsimd.dma_start(tmp[0::2, :], qs[1::2, :])
                    nc.gpsimd.dma_start(tmp[1::2, :], qs[0::2, :])
                    t1 = rt_pool.tile([P, N], bf16, name="t1")
                    nc.vector.tensor_mul(t1[:], qs[:], cosp[:])
                    t2 = rt_pool.tile([P, N], bf16, name="t2")
                    nc.vector.tensor_mul(t2[:], tmp[:], sinp[:])
                    nc.vector.tensor_add(qk[f][:], t1[:], t2[:])

                for h in (2 * p, 2 * p + 1):
                    po = (h % 2) * D
                    qT = qk[p][po:po + D, :]
                    kT = qk[8 + p][po:po + D, :]
                    vh = [v_aug[mt].rearrange("p (h e) -> p h e",
                                              e=D + 1)[:, h, :]
                          for mt in range(NT)]
                    et = [et_pool.tile([P, N], bf16, name=f"et{mt}",
                                       tag=f"et{mt}") for mt in range(NT)]
                    for mt in range(NT):
                        ps = pm.tile([P, N], f32, name="psS", tag="mm", bufs=2)
                        for ch in range(2):
                            nc.tensor.matmul(
                                ps[:, ch * 512:(ch + 1) * 512],
                                lhsT=kT[:, mt * P:(mt + 1) * P],
                                rhs=qT[:, ch * 512:(ch + 1) * 512],
                                start=True, stop=True,
                            )
                        nc.scalar.activation(et[mt][:], ps[:], AF.Exp,
                                             scale=float(1.0 / np.sqrt(D)))
                    pso = [pm.tile([D + 1, 512], f32, name=f"pso{ch}",
                                   tag=f"pso{ch}", bufs=1) for ch in range(2)]
                    for mt in range(NT):
                        for ch in range(2):
                            nc.tensor.matmul(
                                pso[ch][:],
                                lhsT=vh[mt],
                                rhs=et[mt][:, ch * 512:(ch + 1) * 512],
                                start=(mt == 0), stop=(mt == NT - 1),
                            )
                    for ch in range(2):
                        s_row = r_pool.tile([1, 512], f32, name="s_row")
                        nc.vector.tensor_copy(s_row[:], pso[ch][D:D + 1, :])
                        r_row = r_pool.tile([1, 512], f32, name="r_row")
                        nc.vector.reciprocal_approx_fast(r_row[:], s_row[:])
                        rbs = rb_pool.tile([D, 512], f32, name="rbs")
                        nc.gpsimd.partition_broadcast(rbs[:], r_row[:])
                        nc.vector.tensor_tensor(
                            o_fm[h // 2][po:po + D, ch * 512:(ch + 1) * 512],
                            pso[ch][0:D, :], rbs[:], ALU.mult)

            # ---- proj + bias + out ---------------------------------------
            for nt in range(NT):
                ps = pm.tile([P, N], f32, name="psP", tag="mm", bufs=2)
                for ch in range(2):
                    for j in range(CT):
                        nc.tensor.matmul(
                            ps[:, ch * 512:(ch + 1) * 512],
                            lhsT=o_fm[j][:, nt * P:(nt + 1) * P],
                            rhs=wproj_t[j][:, ch * 512:(ch + 1) * 512],
                            start=(j == 0), stop=False,
                        )
                    nc.tensor.matmul(
                        ps[:, ch * 512:(ch + 1) * 512],
                        lhsT=ones_col[:],
                        rhs=bproj_sb[:, ch * 512:(ch + 1) * 512],
                        start=False, stop=True,
                    )
                of = out_pool.tile([P, N], f32, name="of")
                nc.vector.tensor_copy(of[:], ps[:])
                nc.sync.dma_start(out_ext[nt * P:(nt + 1) * P, :], of[:])

    nc.finalize()
    return nc


def _make_in_maps(x, scale, w_qkv, w_proj, b_proj):
    x = np.asarray(x, dtype=np.float32)
    scale = np.asarray(scale, dtype=np.float32)
    w_qkv = np.asarray(w_qkv, dtype=np.float32)
    w_proj = np.asarray(w_proj, dtype=np.float32)
    b_proj = np.asarray(b_proj, dtype=np.float32)

    # fold the RMSNorm scale into w_qkv (exact when scale == 1)
    w_eff = (scale[:, None] * w_qkv).astype(BF16)
    # pre-tile the q|k columns: [16, 8, 128, 128]
    wqk = np.ascontiguousarray(
        w_eff[:, :2 * C].reshape(CT, P, 16, P).transpose(2, 0, 1, 3))
    wv = np.ascontiguousarray(w_eff[:, 2 * C:])
    wproj_b = w_proj.astype(BF16)
    bproj_b = b_proj.reshape(1, C).astype(BF16)
    cosp, sinp = _rope_tables()

    in_maps = []
    for i in range(B):
        in_maps.append({
            "x": np.ascontiguousarray(x[i]).astype(BF16),
            "wqk": wqk,
            "wv": wv,
            "wproj": wproj_b,
            "bproj": bproj_b,
            "cosp": cosp.astype(BF16),
            "sinp": sinp.astype(BF16),
        })
    return in_maps


def _run(inputs, trace=False):
    from concourse import bacc
    from concourse.bass_utils import run_bass_kernel_spmd

    nc = build(bacc.Bacc())
    in_maps = _make_in_maps(**inputs)
    res = run_bass_kernel_spmd(nc, in_maps, list(range(B)), trace=trace)
    out = np.stack([np.asarray(res.results[i]["out"], dtype=np.float32)
                    for i in range(B)], axis=0)
    return out, res


def kernel(x, scale, w_qkv, w_proj, b_proj):
    out, _ = _run(dict(x=x, scale=scale, w_qkv=w_qkv, w_proj=w_proj,
                       b_proj=b_proj))
    return out


# revision 38
# speedup vs baseline: 1.2749x; 1.2749x over previous
"""Fused attention block (RMSNorm -> QKV -> 2D RoPE -> SDPA -> proj) on 8
Trainium2 NeuronCores, data-parallel over the batch dimension (B=8, one batch
element per core; no collectives).

Self-contained: hardcodes shapes B=8, N=1024, C=1024, H=16, D=64.
"""

import sys
import types

import numpy as np
import ml_dtypes

B = 8
N = 1024
C = 1024
H = 16
D = 64
GRID = 32
EPS = 1e-6
P = 128
NT = N // P      # 8 token tiles
CT = C // P      # 8 channel tiles
HPT = P // D     # heads per feature tile (2)

BF16 = ml_dtypes.bfloat16


def _rope_tables():
    """Feature-major RoPE tables [64, N] duplicated to [128, N].

    cosp[p, n] = cos(angle[n, (p % 64) // 2])
    sinp[p, n] = -sin(...) for even (p%64), +sin(...) for odd.
    """
    quarter = D // 4
    freqs = 1.0 / (10000.0 ** (np.arange(quarter, dtype=np.float64) / quarter))
    ys, xs = np.meshgrid(np.arange(GRID, dtype=np.float64),
                         np.arange(GRID, dtype=np.float64), indexing='ij')
    ang_x = xs.reshape(-1)[:, None] * freqs[None, :]      # [N, 16]
    ang_y = ys.reshape(-1)[:, None] * freqs[None, :]      # [N, 16]
    angles = np.concatenate([ang_x, ang_y], axis=-1)      # [N, 32]
    cos = np.cos(angles)                                  # [N, 32]
    sin = np.sin(angles)
    # expand pair index to head-dim d: factor for d is table[d//2]
    cos_d = np.repeat(cos, 2, axis=1).T                   # [64, N]
    sin_d = np.repeat(sin, 2, axis=1).T                   # [64, N]
    sign = np.where(np.arange(D) % 2 == 0, -1.0, 1.0)[:, None]
    sin_d = sin_d * sign                                  # [64, N] sign-folded
    cosp = np.concatenate([cos_d, cos_d], axis=0)         # [128, N]
    sinp = np.concatenate([sin_d, sin_d], axis=0)
    return cosp.astype(np.float32), sinp.astype(np.float32)


def build(nc):
    import concourse.mybir as mybir
    import concourse.tile as tile
    from concourse.masks import make_identity

    f32 = mybir.dt.float32
    bf16 = mybir.dt.bfloat16
    AF = mybir.ActivationFunctionType
    ALU = mybir.AluOpType

    x_ext = nc.declare_dram_parameter("x", [N, C], f32, isOutput=False)
    wqkv_ext = nc.declare_dram_parameter("wqkv", [C, 3 * C], bf16, isOutput=False)
    wproj_ext = nc.declare_dram_parameter("wproj", [C, C], bf16, isOutput=False)
    bproj_ext = nc.declare_dram_parameter("bproj", [1, C], bf16, isOutput=False)
    cosp_ext = nc.declare_dram_parameter("cosp", [P, N], bf16, isOutput=False)
    sinp_ext = nc.declare_dram_parameter("sinp", [P, N], bf16, isOutput=False)
    out_ext = nc.declare_dram_parameter("out", [N, C], f32, isOutput=True)

    with tile.TileContext(nc) as tc:
        # ---- persistent pools --------------------------------------------
        with (
            tc.tile_pool(name="consts", bufs=1) as consts,
            tc.tile_pool(name="wproj", bufs=1) as wproj_pool,
            tc.tile_pool(name="tabs", bufs=1) as tabs,
            tc.tile_pool(name="xnt", bufs=1) as xnt_pool,
            tc.tile_pool(name="qk", bufs=1) as qk_pool,
            tc.tile_pool(name="vaug", bufs=1) as v_pool,
            tc.tile_pool(name="osb", bufs=1) as o_pool,
        ):
            ident = consts.tile([P, P], bf16, name="ident")
            make_identity(nc, ident[:])
            ones_col = consts.tile([1, P], bf16, name="ones_col")
            nc.vector.memset(ones_col[:], 1.0)
            eps_t = consts.tile([P, 1], f32, name="eps_t")
            nc.vector.memset(eps_t[:], EPS)
            bproj_sb = consts.tile([1, C], bf16, name="bproj_sb")
            nc.scalar.dma_start(bproj_sb[:], bproj_ext[:])
            cosp = tabs.tile([P, N], bf16, name="cosp")
            sinp = tabs.tile([P, N], bf16, name="sinp")
            nc.scalar.dma_start(cosp[:], cosp_ext[:])
            nc.scalar.dma_start(sinp[:], sinp_ext[:])
            wproj_t = [wproj_pool.tile([P, C], bf16, name=f"wp{j}", tag=f"wp{j}")
                       for j in range(CT)]
            for j in range(CT):
                nc.scalar.dma_start(wproj_t[j][:],
                                    wproj_ext[j * P:(j + 1) * P, :])

            xnt = [xnt_pool.tile([P, N], bf16, name=f"xnt{j}", tag=f"xnt{j}")
                   for j in range(CT)]
            # roped q/k, feature-major: tiles f=0..7 -> q cols, 8..15 -> k cols
            qk = [qk_pool.tile([P, N], bf16, name=f"qk{f}", tag=f"qk{f}")
                  for f in range(16)]
            # v token-major with ones column: [128, 16*65]
            v_aug = [v_pool.tile([P, H * (D + 1)], bf16, name=f"vaug{i}",
                                 tag=f"vaug{i}") for i in range(NT)]
            # attention output, feature-major like qk: tile f holds heads 2f,2f+1
            o_fm = [o_pool.tile([P, N], bf16, name=f"ofm{f}", tag=f"ofm{f}")
                    for f in range(CT)]

            # ---- phase A+B: norm, transpose, QKV (wqkv scoped) -----------
            with (
                tc.tile_pool(name="wqkv", bufs=1) as wq_pool,
                tc.tile_pool(name="xin", bufs=2) as x_pool,
                tc.tile_pool(name="stats", bufs=2) as st_pool,
                tc.tile_pool(name="sq", bufs=2) as sq_pool,
                tc.tile_pool(name="xn", bufs=2) as xn_pool,
                tc.tile_pool(name="pst", bufs=4, space="PSUM") as psA,
                tc.tile_pool(name="psqkv", bufs=2, space="PSUM") as psB,
                tc.tile_pool(name="qsev", bufs=2) as qs_pool,
                tc.tile_pool(name="tmp", bufs=2) as tmp_pool,
                tc.tile_pool(name="ropet", bufs=2) as rt_pool,
            ):
                wqkv_t = [wq_pool.tile([P, 3 * C], bf16, name=f"wq{j}",
                                       tag=f"wq{j}") for j in range(CT)]
                for j in range(CT):
                    nc.scalar.dma_start(wqkv_t[j][:],
                                        wqkv_ext[j * P:(j + 1) * P, :])

                # norm + transpose -> xnt
                for i in range(NT):
                    x_i = x_pool.tile([P, C], f32, name="x_i")
                    nc.sync.dma_start(x_i[:], x_ext[i * P:(i + 1) * P, :])
                    sq = sq_pool.tile([P, C], bf16, name="sq")
                    ssq = st_pool.tile([P, 1], f32, name="ssq")
                    nc.scalar.activation(sq[:], x_i[:], AF.Square,
                                         accum_out=ssq[:])
                    rms = st_pool.tile([P, 1], f32, name="rms")
                    nc.scalar.activation(rms[:], ssq[:], AF.Sqrt,
                                         scale=1.0 / C, bias=eps_t[:])
                    rs = st_pool.tile([P, 1], f32, name="rs")
                    nc.vector.reciprocal(rs[:], rms[:])
                    xn = xn_pool.tile([P, C], bf16, name="xn")
                    nc.vector.tensor_scalar(xn[:], x_i[:], rs[:], None, ALU.mult)
                    for j in range(CT):
                        pt = psA.tile([P, P], bf16, name="pt")
                        nc.tensor.transpose(pt[:], xn[:, j * P:(j + 1) * P],
                                            ident[:])
                        nc.vector.tensor_copy(xnt[j][:, i * P:(i + 1) * P], pt[:])

                # v token-major: lhsT = xnT block (stationary), rhs = w_v cols
                for i in range(NT):
                    for ch in range(2):
                        ps = psB.tile([P, 512], f32, name="psv", tag="psv")
                        for j in range(CT):
                            nc.tensor.matmul(
                                ps[:],
                                lhsT=xnt[j][:, i * P:(i + 1) * P],
                                rhs=wqkv_t[j][:, 2 * C + ch * 512:
                                              2 * C + (ch + 1) * 512],
                                start=(j == 0), stop=(j == CT - 1),
                            )
                        dst = v_aug[i].rearrange("p (h e) -> p h e", e=D + 1)
                        dst = dst[:, ch * 8:(ch + 1) * 8, 0:D]
                        src = ps.rearrange("p (h d) -> p h d", d=D)
                        nc.scalar.activation(dst, src, AF.Copy)
                    ones_dst = v_aug[i].rearrange("p (h e) -> p h e", e=D + 1)
                    nc.vector.memset(ones_dst[:, :, D:D + 1], 1.0)

                # q/k feature-major + RoPE; interleave q/k tiles so head
                # pairs complete early (attention overlaps this phase)
                f_order = []
                for p in range(CT):
                    f_order += [p, 8 + p]
                for f in f_order:
                    qs = qs_pool.tile([P, N], bf16, name="qs")
                    for ch in range(2):
                        ps = psB.tile([P, 512], f32, name="psqk", tag="psqk")
                        for j in range(CT):
                            nc.tensor.matmul(
                                ps[:],
                                lhsT=wqkv_t[j][:, f * P:(f + 1) * P],
                                rhs=xnt[j][:, ch * 512:(ch + 1) * 512],
                                start=(j == 0), stop=(j == CT - 1),
                            )
                        nc.scalar.activation(qs[:, ch * 512:(ch + 1) * 512],
                                             ps[:], AF.Copy)
                    tmp = tmp_pool.tile([P, N], bf16, name="tmp")
                    nc.sync.dma_start(tmp[0::2, :], qs[1::2, :])
                    nc.sync.dma_start(tmp[1::2, :], qs[0::2, :])
                    t1 = rt_pool.tile([P, N], bf16, name="t1")
                    nc.vector.tensor_mul(t1[:], qs[:], cosp[:])
                    t2 = rt_pool.tile([P, N], bf16, name="t2")
                    nc.vector.tensor_mul(t2[:], tmp[:], sinp[:])
                    nc.vector.tensor_add(qk[f][:], t1[:], t2[:])

            # ---- phase C: attention per head -----------------------------
            with (
                tc.tile_pool(name="et", bufs=2) as et_pool,
                tc.tile_pool(name="psS", bufs=2, space="PSUM") as psS,
                tc.tile_pool(name="psO", bufs=2, space="PSUM") as psO,
                tc.tile_pool(name="rstat", bufs=4) as r_pool,
                tc.tile_pool(name="rbc", bufs=4) as rb_pool,
            ):
                for h in range(H):
                    po = (h % 2) * D
                    qT = qk[h // 2][po:po + D, :]
                    kT = qk[8 + h // 2][po:po + D, :]
                    vh = [v_aug[mt].rearrange("p (h e) -> p h e",
                                              e=D + 1)[:, h, :]
                          for mt in range(NT)]
                    et = [et_pool.tile([P, N], bf16, name=f"et{mt}",
                                       tag=f"et{mt}") for mt in range(NT)]
                    for mt in range(NT):
                        ps = psS.tile([P, N], f32, name="psS", tag="psS")
                        for ch in range(2):
                            nc.tensor.matmul(
                                ps[:, ch * 512:(ch + 1) * 512],
                                lhsT=kT[:, mt * P:(mt + 1) * P],
                                rhs=qT[:, ch * 512:(ch + 1) * 512],
                                start=True, stop=True,
                            )
                        nc.scalar.activation(et[mt][:], ps[:], AF.Exp,
                                             scale=float(1.0 / np.sqrt(D)))
                    # A@V feature-major: oT_aug[d(+s), n] accumulated over mt
                    pso = psO.tile([D + 1, N], f32, name="pso", tag="pso")
                    for mt in range(NT):
                        for ch in range(2):
                            nc.tensor.matmul(
                                pso[:, ch * 512:(ch + 1) * 512],
                                lhsT=vh[mt],
                                rhs=et[mt][:, ch * 512:(ch + 1) * 512],
                                start=(mt == 0), stop=(mt == NT - 1),
                            )
                    for ch in range(2):
                        s_row = r_pool.tile([1, 512], f32, name="s_row")
                        nc.vector.tensor_copy(
                            s_row[:], pso[D:D + 1, ch * 512:(ch + 1) * 512])
                        r_row = r_pool.tile([1, 512], f32, name="r_row")
                        nc.vector.reciprocal_approx_fast(r_row[:], s_row[:])
                        rbs = rb_pool.tile([D, 512], f32, name="rbs")
                        nc.gpsimd.partition_broadcast(rbs[:], r_row[:])
                        nc.vector.tensor_tensor(
                            o_fm[h // 2][po:po + D, ch * 512:(ch + 1) * 512],
                            pso[0:D, ch * 512:(ch + 1) * 512], rbs[:],
                            ALU.mult)

            # ---- phase D: proj + bias + out ------------------------------
            with (
                tc.tile_pool(name="psP", bufs=4, space="PSUM") as psP,
                tc.tile_pool(name="outst", bufs=4) as out_pool,
            ):
                for nt in range(NT):
                    for ch in range(2):
                        ps = psP.tile([P, 512], f32, name="psP", tag="psP")
                        for j in range(CT):
                            nc.tensor.matmul(
                                ps[:],
                                lhsT=o_fm[j][:, nt * P:(nt + 1) * P],
                                rhs=wproj_t[j][:, ch * 512:(ch + 1) * 512],
                                start=(j == 0), stop=False,
                            )
                        nc.tensor.matmul(
                            ps[:],
                            lhsT=ones_col[:],
                            rhs=bproj_sb[:, ch * 512:(ch + 1) * 512],
                            start=False, stop=True,
                        )
                        of = out_pool.tile([P, 512], f32, name="of")
                        nc.vector.tensor_copy(of[:], ps[:])
                        nc.sync.dma_start(
                            out_ext[nt * P:(nt + 1) * P,
                                    ch * 512:(ch + 1) * 512], of[:])

    nc.finalize()
    return nc


def _make_in_maps(x, scale, w_qkv, w_proj, b_proj):
    x = np.asarray(x, dtype=np.float32)
    scale = np.asarray(scale, dtype=np.float32)
    w_qkv = np.asarray(w_qkv, dtype=np.float32)
    w_proj = np.asarray(w_proj, dtype=np.float32)
    b_proj = np.asarray(b_proj, dtype=np.float32)

    # fold the RMSNorm scale into w_qkv (exact when scale == 1)
    w_eff = (scale[:, None] * w_qkv).astype(BF16)
    wproj_b = w_proj.astype(BF16)
    bproj_b = b_proj.reshape(1, C).astype(BF16)
    cosp, sinp = _rope_tables()
    cosp_b = cosp.astype(BF16)
    sinp_b = sinp.astype(BF16)

    in_maps = []
    for i in range(B):
        in_maps.append({
            "x": np.ascontiguousarray(x[i]),
            "wqkv": w_eff,
            "wproj": wproj_b,
            "bproj": bproj_b,
            "cosp": cosp_b,
            "sinp": sinp_b,
        })
    return in_maps


def _run(inputs, trace=False):
    from concourse import bacc
    from concourse.bass_utils import run_bass_kernel_spmd

    nc = build(bacc.Bacc())
    in_maps = _make_in_maps(**inputs)
    res = run_bass_kernel_spmd(nc, in_maps, list(range(B)), trace=trace)
    out = np.stack([np.asarray(res.results[i]["out"], dtype=np.float32)
                    for i in range(B)], axis=0)
    return out, res


def kernel(x, scale, w_qkv, w_proj, b_proj):
    out, _ = _run(dict(x=x, scale=scale, w_qkv=w_qkv, w_proj=w_proj,
                       b_proj=b_proj))
    return out
